# revision 1
# baseline (speedup 1.0000x reference)
"""Trainium2 Bass kernel for nn_DCTFeatureModel.

Math: the reference pipeline (3D DCT-II over [time-in-bin, H, W], mean over
DCT bins, full-receptive-field Conv3d, bias, LeakyReLU) is linear up to the
LeakyReLU, so everything folds into a single small matmul:

    feat[b,s,o] = LeakyReLU( sum_{c,t,i,j} x[b,s,c,t,i,j] * Weff[s,o,t,i,j]
                             + bias[s,o] )
    Weff[s,o,t,i,j] = (1/8) * sum_{f,p,q} Ct[f,t] Cs[p,i] Cs[q,j] W[s,o,f,p,q]

Weff is tiny (2*64*2048 floats) and computed on host. The device kernel is
memory-bound: stream x (134 MB full / 16.8 MB per core), reduce over the 8
DCT bins (c), then a [128b x 2048k] @ [2048k x 64o] matmul per subwindow.

Device dataflow (per core, fp32 exact): the host lays each core's x shard
out as contiguous [kin=128, chin*b = 1024] blocks per (s, c, g) so every
DMA unit is a fully contiguous 512 KB load arriving with the contraction
index already on partitions, and all 8 c-blocks of a (s, g) group land
within a ~10 us window. The c-reduction is a binary tree per group split
across DVE and GPSIMD (each engine's chain kept engine-local, one cross
join at the root); each reduced tile's 128-col slices are directly the
transposed matmul operands (no PE transposes, no PSUM->SBUF copies).
PE does 32 accumulating fp32 matmuls + 2 rank-1 bias matmuls;
LeakyReLU = max(v, 0.02v) on DVE.

Sharding: pure data-parallel over batch, 1024/8 = 128 rows per core.
"""

from contextlib import ExitStack

import numpy as np

import concourse.bacc as bacc
import concourse.tile as tile
from concourse import mybir
from concourse.bass_utils import run_bass_kernel_spmd

# Problem shapes (hardcoded per contract)
B = 1024
NCORES = 8
BS = B // NCORES          # 128 batch rows per core
NSW = 2                   # subwindows
NBINS = 8                 # DCT bins (mean-reduced)
NDCT = 32                 # time points per bin
HW = 8
NF = 64                   # conv output filters per subwindow
K = NDCT * HW * HW        # 2048 contraction elements per (s, c)
P = 128                   # partitions
NCHUNK = K // P           # 16 k-chunks of 128
NG = 2                    # chunk-groups per s
CPG = NCHUNK // NG        # 8 chunks per group
GW = CPG * P              # 1024 columns per group tile
OUT_F = NSW * NF          # 128 output features
SLOPE = 0.02

F32 = mybir.dt.float32

_cached = None
last_results = None


def _dct2(N):
    n = np.arange(N, dtype=np.float64)
    k = np.arange(N, dtype=np.float64)
    return 2.0 * np.cos(np.pi * (2.0 * n[None, :] + 1.0) * k[:, None] / (2.0 * N))


def _kernel_body(tc, x, w, bias, out):
    """x: [NSW*NBINS*NG, 128, GW] (s,c,g blocks, each [kin, chin*b], contiguous)
    w: [P, NSW*NCHUNK*NF]; bias: [1, OUT_F]; out: [BS, OUT_F]"""
    nc = tc.nc
    with ExitStack() as ctx:
        const_pool = ctx.enter_context(tc.tile_pool(name="const", bufs=1))
        xpool = ctx.enter_context(tc.tile_pool(name="xp", bufs=14))
        tpool = ctx.enter_context(tc.tile_pool(name="tp", bufs=8))
        zpool = ctx.enter_context(tc.tile_pool(name="zp", bufs=3))
        opool = ctx.enter_context(tc.tile_pool(name="op", bufs=1))
        pft_pool = ctx.enter_context(tc.tile_pool(name="pft", bufs=1, space="PSUM"))

        # consts dispatched off the sync engine so the x stream starts at once
        w_sb = const_pool.tile([P, NSW * NCHUNK * NF], F32)
        nc.scalar.dma_start(out=w_sb, in_=w)
        bias_sb = const_pool.tile([1, OUT_F], F32)
        nc.scalar.dma_start(out=bias_sb, in_=bias)
        ones = const_pool.tile([1, P], F32)
        nc.gpsimd.memset(ones, 1.0)

        out_sb = opool.tile([BS, OUT_F], F32)
        psum_feat = [
            pft_pool.tile([P, NF], F32, tag=f"feat{s}", name=f"psum_feat{s}")
            for s in range(NSW)
        ]

        for s in range(NSW):
            for g in range(NG):
                # --- load the 8 c-slices of this (s, g): contiguous [128, 1024] ---
                tiles = []
                for c in range(NBINS):
                    t = xpool.tile([P, GW], F32, tag="x", name=f"x_{s}_{g}_{c}")
                    nc.sync.dma_start(out=t, in_=x[(s * NBINS + c) * NG + g])
                    tiles.append(t)
                # --- binary tree c-reduction on DVE + GPSIMD ---
                # lvl0: (01)->DVE (23)->GP (45)->GP (67)->DVE   (67 gates the tail)
                l0 = []
                engs = [nc.vector, nc.gpsimd, nc.gpsimd, nc.vector]
                for j in range(4):
                    u = tpool.tile([P, GW], F32, tag="tree", name=f"t_{s}_{g}_{j}")
                    engs[j].tensor_add(out=u, in0=tiles[2 * j], in1=tiles[2 * j + 1])
                    l0.append(u)
                v0 = tpool.tile([P, GW], F32, tag="tree", name=f"v0_{s}_{g}")
                nc.gpsimd.tensor_add(out=v0, in0=l0[0], in1=l0[1])
                v1 = tpool.tile([P, GW], F32, tag="tree", name=f"v1_{s}_{g}")
                nc.vector.tensor_add(out=v1, in0=l0[2], in1=l0[3])
                z = zpool.tile([P, GW], F32, tag="z", name=f"z_{s}_{g}")
                nc.vector.tensor_add(out=z, in0=v0, in1=v1)

                # --- PE: each 128-col slice of z is a ready lhsT chunk ---
                for j in range(CPG):
                    ch = g * CPG + j
                    nc.tensor.matmul(
                        psum_feat[s],
                        lhsT=z[:, j * P:(j + 1) * P],
                        rhs=w_sb[:, (s * NCHUNK + ch) * NF:(s * NCHUNK + ch + 1) * NF],
                        start=(ch == 0),
                        stop=False,
                    )

        for s in range(NSW):
            # bias via rank-1 matmul: ones[1, b].T @ bias[1, o]
            nc.tensor.matmul(
                psum_feat[s],
                lhsT=ones,
                rhs=bias_sb[:, s * NF:(s + 1) * NF],
                start=False,
                stop=True,
            )
            # LeakyReLU(v) = max(v, slope*v)  (slope < 1)
            tmp = tpool.tile([P, NF], F32, tag="lrelu", name=f"lr_{s}")
            nc.vector.tensor_scalar_mul(tmp, psum_feat[s], SLOPE)
            nc.vector.tensor_max(
                out=out_sb[:, s * NF:(s + 1) * NF], in0=psum_feat[s], in1=tmp
            )

        nc.sync.dma_start(out=out, in_=out_sb)


def _build():
    global _cached
    if _cached is not None:
        return _cached
    nc = bacc.Bacc(
        "TRN2",
        target_bir_lowering=False,
        debug=False,
        enable_asserts=False,
        num_devices=NCORES,
    )
    x_ap = nc.dram_tensor(
        "x", [NSW * NBINS * NG, P, GW], F32, kind="ExternalInput"
    ).ap()
    w_ap = nc.dram_tensor("w", [P, NSW * NCHUNK * NF], F32, kind="ExternalInput").ap()
    b_ap = nc.dram_tensor("bias", [1, OUT_F], F32, kind="ExternalInput").ap()
    out_ap = nc.dram_tensor("out", [BS, OUT_F], F32, kind="ExternalOutput").ap()
    with tile.TileContext(nc, trace_sim=False) as tc:
        _kernel_body(tc, x_ap, w_ap, b_ap, out_ap)
    nc.compile()
    _cached = nc
    return nc


def kernel(x, W, b):
    global last_results
    assert x.shape == (B, 1, NSW * NBINS * NDCT, HW, HW), x.shape
    nc = _build()

    # Host-side folding of the DCT matrices into the conv weights (tiny).
    Ct = _dct2(NDCT)                       # [f, t]
    Cs = _dct2(HW)                         # [p, i]
    Weff = np.einsum(
        "ft,pi,qj,sofpq->sotij", Ct, Cs, Cs, W.astype(np.float64), optimize=True
    ) / float(NBINS)
    Weff_k = Weff.reshape(NSW, NF, K)      # [s, o, k]
    # device layout: w[p, s*NCHUNK*NF + ch*NF + o] = Weff_k[s, o, ch*128 + p]
    w_dev = np.ascontiguousarray(
        Weff_k.reshape(NSW, NF, NCHUNK, P).transpose(3, 0, 2, 1).reshape(P, NSW * NCHUNK * NF)
    ).astype(np.float32)
    bias_dev = np.ascontiguousarray(b.reshape(1, OUT_F)).astype(np.float32)

    x2 = x.reshape(B, NSW, NBINS, NG, CPG, P)  # (b, s, c, g, chin, kin)
    in_maps = []
    for i in range(NCORES):
        xs = x2[i * BS:(i + 1) * BS]
        # -> [s, c, g, kin, chin, b]: one contiguous [128, 1024] block per (s,c,g)
        xt = np.ascontiguousarray(xs.transpose(1, 2, 3, 5, 4, 0)).reshape(
            NSW * NBINS * NG, P, GW
        )
        in_maps.append({"x": xt, "w": w_dev, "bias": bias_dev})
    res = run_bass_kernel_spmd(nc, in_maps, core_ids=list(range(NCORES)))
    last_results = res
    return np.concatenate([r["out"] for r in res.results], axis=0)



# revision 7
# speedup vs baseline: 1.0927x; 1.0927x over previous
"""Trainium2 Bass kernel for nn_DCTFeatureModel.

Math: the reference pipeline (3D DCT-II over [time-in-bin, H, W], mean over
DCT bins, full-receptive-field Conv3d, bias, LeakyReLU) is linear up to the
LeakyReLU, so everything folds into a single small matmul:

    feat[b,s,o] = LeakyReLU( sum_{c,k} x[b,s,c,k] * Weff[s,o,k] + bias[s,o] )
    Weff[s,o,k=(t,i,j)] = (1/8) * sum_{f,p,q} Ct[f,t] Cs[p,i] Cs[q,j] W[s,o,f,p,q]

Weff is tiny (2*64*2048 floats) and computed on host. The device kernel is
memory-bound: stream x (134 MB full / 16.8 MB per core) once from HBM.

Device dataflow (per core): host lays the x shard out as 8 contiguous 2MB
blocks [kin=128, ch4=4, c=8, b=128] (one per (s, chunk-quad)), so the whole
shard is 8 back-to-back 2MB DMAs into 8 resident SBUF tiles (128 KB of the
208 KB per partition) with zero buffer-reuse waits -- the DMA queue streams
at HBM line rate start to finish. The PE consumes each tile directly with
w-stationary float32r matmuls (moving dim 512 -> 1 cycle/row): for each of
the 16 k-chunks per subwindow, out[o, (c,b)] += Wchunk.T @ xchunk, keeping
the 8 DCT bins as separate PSUM columns ([64, 1024] per subwindow = 2
banks). No vector-engine reduction tree at all. DVE then folds the 8 c
columns (3 adds), and bias (applied as a rank-1 matmul of bias/8 into every
c block) + LeakyReLU finish the [64, 128] result per subwindow.

Sharding: pure data-parallel over batch, 1024/8 = 128 rows per core.
"""

from contextlib import ExitStack

import numpy as np

import concourse.bacc as bacc
import concourse.tile as tile
from concourse import mybir
from concourse.bass_utils import run_bass_kernel_spmd

# Problem shapes (hardcoded per contract)
B = 1024
NCORES = 8
BS = B // NCORES          # 128 batch rows per core
NSW = 2                   # subwindows
NBINS = 8                 # DCT bins (mean-reduced)
NDCT = 32                 # time points per bin
HW = 8
NF = 64                   # conv output filters per subwindow
K = NDCT * HW * HW        # 2048 contraction elements per (s, c)
P = 128                   # partitions
NCHUNK = K // P           # 16 k-chunks of 128
QUAD = 4                  # chunks per DMA tile
NQ = NCHUNK // QUAD       # 4 quads per subwindow
TILE_W = QUAD * NBINS * BS  # 4096 cols per x tile: [ch4, c, b]
CW = NBINS * BS           # 1024 cols per chunk: [c, b]
HALF = CW // 2            # 512-column matmul (one PSUM bank)
OUT_F = NSW * NF          # 128 output features
SLOPE = 0.02

F32 = mybir.dt.float32
F32R = mybir.dt.float32r

_cached = None
last_results = None


def _dct2(N):
    n = np.arange(N, dtype=np.float64)
    k = np.arange(N, dtype=np.float64)
    return 2.0 * np.cos(np.pi * (2.0 * n[None, :] + 1.0) * k[:, None] / (2.0 * N))


def _kernel_body(tc, x, w, bias, ones, out):
    """x: [NSW*NQ, P, TILE_W] (s-major quads, cols [ch4, c, b], contiguous)
    w: [P, NSW*NCHUNK*NF]; bias: [1, OUT_F] (pre-divided by NBINS);
    out: [NF, NSW*BS] (cols [s, b]; host transposes)."""
    nc = tc.nc
    with ExitStack() as ctx:
        const_pool = ctx.enter_context(tc.tile_pool(name="const", bufs=1))
        xpool = ctx.enter_context(tc.tile_pool(name="xp", bufs=NSW * NQ))
        spool = ctx.enter_context(tc.tile_pool(name="sp", bufs=3))
        opool = ctx.enter_context(tc.tile_pool(name="op", bufs=1))
        ppool = ctx.enter_context(tc.tile_pool(name="pp", bufs=1, space="PSUM"))

        # consts off the scalar queue so the x stream owns the sync queue
        w_sb = const_pool.tile([P, NSW * NCHUNK * NF], F32R)
        nc.scalar.dma_start(out=w_sb, in_=w)
        bias_sb = const_pool.tile([1, OUT_F], F32R)
        nc.scalar.dma_start(out=bias_sb, in_=bias)
        ones_sb = const_pool.tile([1, HALF], F32R)
        nc.scalar.dma_start(out=ones_sb, in_=ones)

        out_sb = opool.tile([NF, NSW * BS], F32)

        # the full x shard: 8 x 2MB tiles, all DMAs issued up-front, no reuse
        xt = []
        for i in range(NSW * NQ):
            t = xpool.tile([P, TILE_W], F32R, tag="x", name=f"x_{i}")
            nc.sync.dma_start(out=t, in_=x[i])
            xt.append(t)

        for s in range(NSW):
            # two PSUM banks per subwindow: cols (c, b), c 0-3 and c 4-7
            psA = ppool.tile([NF, HALF], F32, tag=f"pa{s}", name=f"psA{s}")
            psB = ppool.tile([NF, HALF], F32, tag=f"pb{s}", name=f"psB{s}")
            for q in range(NQ):
                t = xt[s * NQ + q]
                for j in range(QUAD):
                    ch = q * QUAD + j
                    wv = w_sb[:, (s * NCHUNK + ch) * NF:(s * NCHUNK + ch + 1) * NF]
                    nc.tensor.matmul(
                        psA,
                        lhsT=wv,
                        rhs=t[:, j * CW:j * CW + HALF],
                        start=(ch == 0),
                        stop=False,
                    )
                    nc.tensor.matmul(
                        psB,
                        lhsT=wv,
                        rhs=t[:, j * CW + HALF:(j + 1) * CW],
                        start=(ch == 0),
                        stop=False,
                    )
            # bias/8 via rank-1 matmul into every (c, b) column
            bv = bias_sb[:, s * NF:(s + 1) * NF]
            nc.tensor.matmul(psA, lhsT=bv, rhs=ones_sb,
                             start=False, stop=True)
            nc.tensor.matmul(psB, lhsT=bv, rhs=ones_sb,
                             start=False, stop=True)

            # fold the 8 DCT bins: [64, 1024] over 2 banks -> [64, 128].
            # DVE may read at most one PSUM operand per op, so chain the adds
            # (ACT does the initial PSUM->SBUF copy in parallel with DVE).
            c01 = spool.tile([NF, 2 * BS], F32, tag="u", name=f"c01_{s}")
            nc.scalar.copy(out=c01, in_=psA[:, :2 * BS])
            a0 = spool.tile([NF, 2 * BS], F32, tag="u", name=f"a0_{s}")
            nc.vector.tensor_add(out=a0, in0=c01, in1=psA[:, 2 * BS:])
            a1 = spool.tile([NF, 2 * BS], F32, tag="u", name=f"a1_{s}")
            nc.vector.tensor_add(out=a1, in0=a0, in1=psB[:, :2 * BS])
            a2 = spool.tile([NF, 2 * BS], F32, tag="u", name=f"a2_{s}")
            nc.vector.tensor_add(out=a2, in0=a1, in1=psB[:, 2 * BS:])
            r = spool.tile([NF, BS], F32, tag="r", name=f"r_{s}")
            nc.vector.tensor_add(out=r, in0=a2[:, :BS], in1=a2[:, BS:])
            # LeakyReLU(v) = max(v, slope*v)  (slope < 1)
            tmp = spool.tile([NF, BS], F32, tag="r", name=f"lr_{s}")
            nc.vector.tensor_scalar_mul(tmp, r, SLOPE)
            nc.vector.tensor_max(out=out_sb[:, s * BS:(s + 1) * BS], in0=r, in1=tmp)

        nc.scalar.dma_start(out=out, in_=out_sb)


def _build():
    global _cached
    if _cached is not None:
        return _cached
    nc = bacc.Bacc(
        "TRN2",
        target_bir_lowering=False,
        debug=False,
        enable_asserts=False,
        num_devices=NCORES,
    )
    x_ap = nc.dram_tensor("x", [NSW * NQ, P, TILE_W], F32R, kind="ExternalInput").ap()
    w_ap = nc.dram_tensor("w", [P, NSW * NCHUNK * NF], F32R, kind="ExternalInput").ap()
    b_ap = nc.dram_tensor("bias", [1, OUT_F], F32R, kind="ExternalInput").ap()
    ones_ap = nc.dram_tensor("ones", [1, HALF], F32R, kind="ExternalInput").ap()
    out_ap = nc.dram_tensor("out", [NF, NSW * BS], F32, kind="ExternalOutput").ap()
    with tile.TileContext(nc, trace_sim=False) as tc:
        _kernel_body(tc, x_ap, w_ap, b_ap, ones_ap, out_ap)
    nc.compile()
    _cached = nc
    return nc


def kernel(x, W, b):
    global last_results
    assert x.shape == (B, 1, NSW * NBINS * NDCT, HW, HW), x.shape
    nc = _build()

    # Host-side folding of the DCT matrices into the conv weights (tiny).
    Ct = _dct2(NDCT)                       # [f, t]
    Cs = _dct2(HW)                         # [p, i]
    Weff = np.einsum(
        "ft,pi,qj,sofpq->sotij", Ct, Cs, Cs, W.astype(np.float64), optimize=True
    ) / float(NBINS)
    Weff_k = Weff.reshape(NSW, NF, K)      # [s, o, k]
    # device layout: w[p, (s*NCHUNK + ch)*NF + o] = Weff_k[s, o, ch*128 + p]
    w_dev = np.ascontiguousarray(
        Weff_k.reshape(NSW, NF, NCHUNK, P).transpose(3, 0, 2, 1).reshape(P, NSW * NCHUNK * NF)
    ).astype(np.float32)
    bias_dev = np.ascontiguousarray(
        (b.astype(np.float64) / NBINS).reshape(1, OUT_F)
    ).astype(np.float32)

    # x[b, 0, t_global, i, j]; t_global = s*256 + c*32 + t; k = t*64 + ij,
    # chunk ch = k // 128, kin = k % 128 = (t % 2)*64 + ij
    x2 = x.reshape(B, NSW, NBINS, NCHUNK, 2, HW * HW)  # (b, s, c, ch, th, ij)
    in_maps = []
    for i in range(NCORES):
        xs = x2[i * BS:(i + 1) * BS]
        # -> [s, q, kin=(th,ij), ch4, c, b]: contiguous [128, 4096] per (s, q)
        xt = np.ascontiguousarray(
            xs.transpose(1, 3, 4, 5, 2, 0)      # [s, ch, th, ij, c, b]
            .reshape(NSW, NQ, QUAD, P, NBINS, BS)
            .transpose(0, 1, 3, 2, 4, 5)        # [s, q, kin, ch4, c, b]
        ).reshape(NSW * NQ, P, TILE_W)
        in_maps.append({"x": xt, "w": w_dev, "bias": bias_dev,
                        "ones": np.ones((1, HALF), dtype=np.float32)})
    res = run_bass_kernel_spmd(nc, in_maps, core_ids=list(range(NCORES)))
    last_results = res
    # out[o, s*BS + b] -> feat[b, s*NF + o]
    return np.ascontiguousarray(
        np.concatenate(
            [r["out"].reshape(NF, NSW, BS).transpose(2, 1, 0) for r in res.results],
            axis=0,
        ).reshape(B, OUT_F)
    )
